# revision 1
# baseline (speedup 1.0000x reference)
"""MultiHeadAttention kernel for 8x TRN2 NeuronCores.

The reference module's einsum reduces the attention tensor over BOTH the
query and key axes (attn_mass = sum_{q,k} softmax(logits)_k), and softmax
rows sum to 1, so attn_mass == Lq exactly for every (batch, head). The
whole computation therefore collapses to

    out = (Lq * (V_heads @ Wv^T + bv)).reshape(N, L, E) @ Wo^T + bo

which is a single dense GEMM after folding the (block-diagonal) per-head
V-projection into the output projection:

    out = V_flat @ W_eff + b_eff
    W_eff[h*hd+a, n] = Lq * sum_b Wv[b, a] * Wo[n, h*hd+b]      (1024 x 1024)
    b_eff[n]         = Lq * sum_{h,b} Wo[n, h*hd+b] * bv[b] + bo[n]

The device kernel is the GEMM, row-sharded across 8 cores (512 rows per
core), computed in TRANSPOSED orientation: out^T[n, m] = sum_k W[k, n]
X[m, k].  Each PSUM bank j holds output columns j*128..(j+1)*128 on
partitions x all 512 rows on the free dim, accumulating lhsT = W-block j
(natural layout) against rhs = X^T k-slabs.  Benefits:

  * bias varies along PARTITIONS -> folded into the PSUM eviction as a
    free per-partition tensor_scalar_add on the vector engine;
  * input stream order [bias+warm | W0+X0 head | X 1-7 | W-blocks 1-7]
    lets bank j finish as soon as W-block j lands, so output DMAs
    overlap the input stream (bulk input DMAs drain through the sync
    engine's FIFO HWDGE queue at ~350 GB/s; the head rides the scalar
    engine's own HWDGE queue to unblock bank 0 early);
  * a few K=128 junk matmuls on real (nonzero!) fp32 data warm the PE
    HAM clock gate before the first real matmul (zero data is
    activity-gated and does not warm the clock; K=1 matmuls do not
    register either).

The host transposes V-shards in, and the (E, RPC) per-core outputs back.
"""

import numpy as np

import concourse.bass as bass
import concourse.bacc as bacc
import concourse.mybir as mybir
from concourse.tile import TileContext
from concourse.bass_utils import run_bass_kernel_spmd

N_CORES = 8
E = 1024            # embed dim == d_model
H, HD = 16, 64      # heads, head dim
ROWS = 4096         # N * L = 2 * 2048
RPC = ROWS // N_CORES   # rows per core = 512
P = 128             # SBUF partitions
KT = E // P         # 8 contraction slabs
JT = E // P         # 8 output-column banks
N_WARM = 11         # K=128 fp32 junk matmuls for PE HAM warm-up

_NC_CACHE = {}
LAST_RESULTS = None  # BassKernelResults of the most recent device run


def _build(dtype):
    f32 = mybir.dt.float32
    nc = bacc.Bacc(None, target_bir_lowering=False)
    # head packs [W-block0 | X-slab0] so one DMA (on the scalar engine's
    # own HWDGE queue, concurrent with the bulk stream) unblocks bank 0.
    head = nc.declare_dram_parameter("head", [P, E + RPC], dtype, isOutput=False)
    xs = nc.declare_dram_parameter("xs", [E, RPC], dtype, isOutput=False)
    wc = nc.declare_dram_parameter("wc", [JT * P, E], dtype, isOutput=False)
    # bw packs bias columns (JT) and a warm-up block (P) side by side.
    bw = nc.declare_dram_parameter("bw", [P, JT + P], f32, isOutput=False)
    outT = nc.declare_dram_parameter("outT", [E, RPC], f32, isOutput=True)

    with TileContext(nc) as tc:
        with (
            tc.tile_pool(name="xp", bufs=1) as xp,
            tc.tile_pool(name="wp", bufs=1) as wp,
            tc.tile_pool(name="bp", bufs=1) as bp,
            tc.tile_pool(name="pp", bufs=1, space="PSUM") as pp,
            tc.tile_pool(name="op", bufs=1) as op,
        ):
            # memset needs no DMA: junk matmuls can start right after the
            # BSP preamble, well before any input data lands.
            wm_t = bp.tile([P, P], f32, name="wm", tag="wm")
            nc.gpsimd.memset(wm_t[:], 1.0)
            bw_t = bp.tile([P, JT + P], f32, name="bw", tag="bw")

            # head [W0|X0] rides the scalar engine's HWDGE queue; the
            # sync queue interleaves W-blocks into the X stream so the
            # PE (fed in data-arrival order below) never starves, with
            # W7 last (only bank 7 trails the stream).  xrank/wrank
            # mirror the FIFO arrival order of each operand.
            # X-priority dual-queue: all of X lands first across BOTH
            # HWDGE queues (head+x1-3 on the scalar queue, x4-7 leading
            # the sync queue), so every bank's k7 unlocks early and the
            # banks then pace off their W-block arrivals, nicely spread.
            head_t = bp.tile([P, E + RPC], dtype, name="head", tag="head")
            nc.scalar.dma_start(out=head_t[:], in_=head[:, :])
            wts = [None] * JT
            wts[0] = head_t[:, 0:E]
            xts = [head_t[:, E:E + RPC]]
            for k in range(1, KT):
                t = xp.tile([P, RPC], dtype, name=f"x{k}", tag=f"x{k}")
                xts.append(t)
            # All of X rides the fast sync queue (bank 0 unblocks ~16us);
            # w1 and w3 ride the slower scalar queue behind the head and
            # land just before banks 1/3 need them; the remaining W
            # blocks follow X on the sync queue with ~2us of margin each.
            for k in range(1, KT):
                nc.sync.dma_start(out=xts[k][:], in_=xs[k * P:(k + 1) * P, :])
            for j in (1, 3):
                wts[j] = wp.tile([P, E], dtype, name=f"w{j}", tag=f"w{j}")
                nc.scalar.dma_start(out=wts[j][:], in_=wc[j * P:(j + 1) * P, :])
            # bias+warm block rides the scalar queue too: off the sync
            # queue's critical X phase, still ~2us ahead of first eviction
            nc.scalar.dma_start(out=bw_t[:], in_=bw[:, :])
            for j in (2, 4, 5, 6):
                wts[j] = wp.tile([P, E], dtype, name=f"w{j}", tag=f"w{j}")
                nc.sync.dma_start(out=wts[j][:], in_=wc[j * P:(j + 1) * P, :])
            # last W block as four separate quarter tiles so bank 7's
            # matmuls chase the quarters as they land
            q = E // 4
            w7q = []
            for c in range(4):
                t = wp.tile([P, q], dtype, name=f"w7q{c}", tag=f"w7q{c}")
                nc.sync.dma_start(
                    out=t[:], in_=wc[(JT - 1) * P:JT * P, c * q:(c + 1) * q]
                )
                w7q.append(t)

            ps = [
                pp.tile([P, RPC], f32, name=f"ps{j}", tag=f"ps{j}")
                for j in range(JT)
            ]

            # PE warm-up on nonzero fp32 data (4 cycles/row -- dense HAM
            # activity) starting right after the preamble, so the HAM
            # clock-gate lifts before the first real matmul.
            for i in range(N_WARM):
                nc.tensor.matmul(
                    ps[i % JT][:, 0:P],
                    wm_t[:, :],
                    wm_t[:, :],
                    start=True,
                    stop=True,
                )

            # Bank-major emission: bank j is gated by its own W block
            # (X has fully landed by then), so banks finish ~evenly
            # spread and their output DMAs overlap the tail.
            def lhsT(j, k):
                if j < JT - 1:
                    return wts[j][:, k * P:(k + 1) * P]
                c = k // 2
                return w7q[c][:, (k - 2 * c) * P:(k - 2 * c + 1) * P]

            for j in range(JT):
                for k in range(KT):
                    nc.tensor.matmul(
                        ps[j],
                        lhsT(j, k),
                        xts[k][:, :],
                        start=(k == 0),
                        stop=(k == KT - 1),
                    )
            for j in range(JT):
                o = op.tile([P, RPC], f32, name=f"o{j}", tag=f"o{j}")
                if j < JT - 1:
                    nc.vector.tensor_scalar_add(o[:], ps[j], bw_t[:, j:j + 1])
                    # HWDGE FIFO: enqueues behind any remaining input
                    # DMAs; only the LAST bank's output is a deadline,
                    # and it issues after the input stream has drained.
                    nc.sync.dma_start(out=outT[j * P:(j + 1) * P, :], in_=o[:])
                else:
                    # halve the final eviction so its first output DMA
                    # overlaps the second half's tensor_scalar_add
                    hh = RPC // 2
                    for c in range(2):
                        nc.vector.tensor_scalar_add(
                            o[:, c * hh:(c + 1) * hh],
                            ps[j][:, c * hh:(c + 1) * hh],
                            bw_t[:, j:j + 1],
                        )
                        nc.sync.dma_start(
                            out=outT[j * P:(j + 1) * P, c * hh:(c + 1) * hh],
                            in_=o[:, c * hh:(c + 1) * hh],
                        )
    nc.compile()
    return nc


def _get_nc(dtype_name):
    if dtype_name not in _NC_CACHE:
        _NC_CACHE[dtype_name] = _build(getattr(mybir.dt, dtype_name))
    return _NC_CACHE[dtype_name]


def _prep_in_maps(V, Wv, bv, Wo, bo, lq):
    V = np.ascontiguousarray(np.asarray(V, dtype=np.float32))
    Wv64 = np.asarray(Wv, np.float64)
    Wo64 = np.asarray(Wo, np.float64)
    bv64 = np.asarray(bv, np.float64)
    bo64 = np.asarray(bo, np.float64)

    # Fold per-head V-projection + output projection + attention mass (== Lq).
    Wo_r = Wo64.reshape(E, H, HD)                       # [n, h, b]
    W_eff = lq * np.einsum("ba,nhb->han", Wv64, Wo_r, optimize=True)
    W_eff = W_eff.reshape(E, E).astype(np.float32)      # [k, n]
    b_eff = (lq * np.einsum("nhb,b->n", Wo_r, bv64) + bo64).astype(np.float32)

    # wc[j*P + p, k*P + c] = W_eff[k*P + p, j*P + c]  (lhsT blocks, natural)
    wc = np.ascontiguousarray(
        W_eff.reshape(KT, P, JT, P).transpose(2, 1, 0, 3).reshape(JT * P, E)
    )
    bw_blk = np.ones((P, JT + P), np.float32)
    bw_blk[:, :JT] = b_eff.reshape(JT, P).T                 # [p, j]

    X = V.reshape(ROWS, E)
    in_maps = []
    for i in range(N_CORES):
        xs_i = np.ascontiguousarray(X[i * RPC:(i + 1) * RPC, :].T)
        head_i = np.empty((P, E + RPC), np.float32)
        head_i[:, :E] = wc[0:P, :]
        head_i[:, E:] = xs_i[0:P, :]
        in_maps.append({"head": head_i, "xs": xs_i, "wc": wc, "bw": bw_blk})
    return in_maps


def kernel(Q, K, V, Wq, bq, Wk, bk, Wv, bv, Wo, bo, dtype_name="float32r", **_unused):
    global LAST_RESULTS
    n, L, e = np.asarray(V).shape
    lq = float(np.asarray(Q).shape[1])
    in_maps = _prep_in_maps(V, Wv, bv, Wo, bo, lq)
    nc = _get_nc(dtype_name)
    LAST_RESULTS = run_bass_kernel_spmd(nc, in_maps, list(range(N_CORES)))
    out = np.concatenate(
        [LAST_RESULTS.results[i]["outT"].T for i in range(N_CORES)], axis=0
    )
    return np.ascontiguousarray(out).reshape(n, L, E)



# revision 7
# speedup vs baseline: 1.1426x; 1.1426x over previous
"""MultiHeadAttention kernel for 8x TRN2 NeuronCores.

The reference module's einsum reduces the attention tensor over BOTH the
query and key axes (attn_mass = sum_{q,k} softmax(logits)_k), and softmax
rows sum to 1, so attn_mass == Lq exactly for every (batch, head). The
whole computation therefore collapses to

    out = (Lq * (V_heads @ Wv^T + bv)).reshape(N, L, E) @ Wo^T + bo

which is a single dense GEMM after folding the (block-diagonal) per-head
V-projection into the output projection:

    out = V_flat @ W_eff + b_eff
    W_eff[h*hd+a, n] = Lq * sum_b Wv[b, a] * Wo[n, h*hd+b]      (1024 x 1024)
    b_eff[n]         = Lq * sum_{h,b} Wo[n, h*hd+b] * bv[b] + bo[n]

The device kernel is the GEMM, row-sharded across 8 cores (512 rows per
core), computed in TRANSPOSED orientation: out^T[n, m] = sum_k W[k, n]
X[m, k].  Each PSUM bank j holds output columns j*128..(j+1)*128 on
partitions x all 512 rows on the free dim, accumulating lhsT = W-block j
(natural layout) against rhs = X^T k-slabs.  Benefits:

  * bias varies along PARTITIONS -> folded into the PSUM eviction as a
    free per-partition tensor_scalar_add on the vector engine;
  * input stream order [bias+warm | W0+X0 head | X 1-7 | W-blocks 1-7]
    lets bank j finish as soon as W-block j lands, so output DMAs
    overlap the input stream (bulk input DMAs drain through the sync
    engine's FIFO HWDGE queue at ~350 GB/s; the head rides the scalar
    engine's own HWDGE queue to unblock bank 0 early);
  * a few K=128 junk matmuls on real (nonzero!) fp32 data warm the PE
    HAM clock gate before the first real matmul (zero data is
    activity-gated and does not warm the clock; K=1 matmuls do not
    register either).

The host transposes V-shards in, and the (E, RPC) per-core outputs back.
"""

import numpy as np
import ml_dtypes

import concourse.bass as bass
import concourse.bacc as bacc
import concourse.mybir as mybir
from concourse.tile import TileContext
from concourse.bass_utils import run_bass_kernel_spmd

N_CORES = 8
E = 1024            # embed dim == d_model
H, HD = 16, 64      # heads, head dim
ROWS = 4096         # N * L = 2 * 2048
RPC = ROWS // N_CORES   # rows per core = 512
P = 128             # SBUF partitions
KT = E // P         # 8 contraction slabs
JT = E // P         # 8 output-column banks
N_WARM = 11         # K=128 fp32 junk matmuls for PE HAM warm-up

_NC_CACHE = {}
LAST_RESULTS = None  # BassKernelResults of the most recent device run


def _build(dtype, n_warm=N_WARM):
    f32 = mybir.dt.float32
    odt = f32 if dtype == mybir.dt.float32r else dtype
    nc = bacc.Bacc(None, target_bir_lowering=False)
    # head packs [W-block0 | X-slab0] so one DMA (on the scalar engine's
    # own HWDGE queue, concurrent with the bulk stream) unblocks bank 0.
    head = nc.declare_dram_parameter("head", [P, E + RPC], dtype, isOutput=False)
    xs = nc.declare_dram_parameter("xs", [E, RPC], dtype, isOutput=False)
    wc = nc.declare_dram_parameter("wc", [JT * P, E], dtype, isOutput=False)
    # bw packs bias columns (JT) and a warm-up block (P) side by side.
    bw = nc.declare_dram_parameter("bw", [P, JT + P], f32, isOutput=False)
    outT = nc.declare_dram_parameter("outT", [E, RPC], odt, isOutput=True)

    with TileContext(nc) as tc:
        with (
            tc.tile_pool(name="xp", bufs=1) as xp,
            tc.tile_pool(name="wp", bufs=1) as wp,
            tc.tile_pool(name="bp", bufs=1) as bp,
            tc.tile_pool(name="pp", bufs=1, space="PSUM") as pp,
            tc.tile_pool(name="op", bufs=1) as op,
        ):
            # memset needs no DMA: junk matmuls can start right after the
            # BSP preamble, well before any input data lands.
            wm_t = bp.tile([P, P], f32, name="wm", tag="wm")
            nc.gpsimd.memset(wm_t[:], 1.0)
            bw_t = bp.tile([P, JT + P], f32, name="bw", tag="bw")

            # head [W0|X0] rides the scalar engine's HWDGE queue; the
            # sync queue interleaves W-blocks into the X stream so the
            # PE (fed in data-arrival order below) never starves, with
            # W7 last (only bank 7 trails the stream).  xrank/wrank
            # mirror the FIFO arrival order of each operand.
            # X-priority dual-queue: all of X lands first across BOTH
            # HWDGE queues (head+x1-3 on the scalar queue, x4-7 leading
            # the sync queue), so every bank's k7 unlocks early and the
            # banks then pace off their W-block arrivals, nicely spread.
            head_t = bp.tile([P, E + RPC], dtype, name="head", tag="head")
            nc.scalar.dma_start(out=head_t[:], in_=head[:, :])
            wts = [None] * JT
            wts[0] = head_t[:, 0:E]
            xts = [head_t[:, E:E + RPC]]
            for k in range(1, KT):
                t = xp.tile([P, RPC], dtype, name=f"x{k}", tag=f"x{k}")
                xts.append(t)
            # All of X rides the fast sync queue (bank 0 unblocks ~16us);
            # w1 and w3 ride the slower scalar queue behind the head and
            # land just before banks 1/3 need them; the remaining W
            # blocks follow X on the sync queue with ~2us of margin each.
            for k in range(1, KT):
                nc.sync.dma_start(out=xts[k][:], in_=xs[k * P:(k + 1) * P, :])
            for j in (1, 3):
                wts[j] = wp.tile([P, E], dtype, name=f"w{j}", tag=f"w{j}")
                nc.scalar.dma_start(out=wts[j][:], in_=wc[j * P:(j + 1) * P, :])
            # bias+warm block rides the scalar queue too: off the sync
            # queue's critical X phase, still ~2us ahead of first eviction
            nc.scalar.dma_start(out=bw_t[:], in_=bw[:, :])
            for j in (2, 4, 5, 6):
                wts[j] = wp.tile([P, E], dtype, name=f"w{j}", tag=f"w{j}")
                nc.sync.dma_start(out=wts[j][:], in_=wc[j * P:(j + 1) * P, :])
            # last W block as four separate quarter tiles so bank 7's
            # matmuls chase the quarters as they land
            q = E // 4
            w7q = []
            for c in range(4):
                t = wp.tile([P, q], dtype, name=f"w7q{c}", tag=f"w7q{c}")
                nc.sync.dma_start(
                    out=t[:], in_=wc[(JT - 1) * P:JT * P, c * q:(c + 1) * q]
                )
                w7q.append(t)

            ps = [
                pp.tile([P, RPC], f32, name=f"ps{j}", tag=f"ps{j}")
                for j in range(JT)
            ]

            # PE warm-up on nonzero fp32 data (4 cycles/row -- dense HAM
            # activity) starting right after the preamble, so the HAM
            # clock-gate lifts before the first real matmul.
            for i in range(n_warm):
                nc.tensor.matmul(
                    ps[i % JT][:, 0:P],
                    wm_t[:, :],
                    wm_t[:, :],
                    start=True,
                    stop=True,
                )

            # Bank-major emission: bank j is gated by its own W block
            # (X has fully landed by then), so banks finish ~evenly
            # spread and their output DMAs overlap the tail.
            def lhsT(j, k):
                if j < JT - 1:
                    return wts[j][:, k * P:(k + 1) * P]
                c = k // 2
                return w7q[c][:, (k - 2 * c) * P:(k - 2 * c + 1) * P]

            for j in range(JT):
                for k in range(KT):
                    nc.tensor.matmul(
                        ps[j],
                        lhsT(j, k),
                        xts[k][:, :],
                        start=(k == 0),
                        stop=(k == KT - 1),
                    )
            for j in range(JT):
                o = op.tile([P, RPC], odt, name=f"o{j}", tag=f"o{j}")
                if j < JT - 1:
                    nc.vector.tensor_scalar_add(o[:], ps[j], bw_t[:, j:j + 1])
                    # HWDGE FIFO: enqueues behind any remaining input
                    # DMAs; only the LAST bank's output is a deadline,
                    # and it issues after the input stream has drained.
                    nc.sync.dma_start(out=outT[j * P:(j + 1) * P, :], in_=o[:])
                else:
                    # halve the final eviction so its first output DMA
                    # overlaps the second half's tensor_scalar_add
                    hh = RPC // 2
                    for c in range(2):
                        nc.vector.tensor_scalar_add(
                            o[:, c * hh:(c + 1) * hh],
                            ps[j][:, c * hh:(c + 1) * hh],
                            bw_t[:, j:j + 1],
                        )
                        nc.sync.dma_start(
                            out=outT[j * P:(j + 1) * P, c * hh:(c + 1) * hh],
                            in_=o[:, c * hh:(c + 1) * hh],
                        )
    nc.compile()
    return nc


def _get_nc(dtype_name, n_warm=N_WARM):
    key = (dtype_name, n_warm)
    if key not in _NC_CACHE:
        _NC_CACHE[key] = _build(getattr(mybir.dt, dtype_name), n_warm)
    return _NC_CACHE[key]


def _prep_in_maps(V, Wv, bv, Wo, bo, lq, np_dtype=np.float32):
    V = np.ascontiguousarray(np.asarray(V, dtype=np.float32))
    Wv64 = np.asarray(Wv, np.float64)
    Wo64 = np.asarray(Wo, np.float64)
    bv64 = np.asarray(bv, np.float64)
    bo64 = np.asarray(bo, np.float64)

    # Fold per-head V-projection + output projection + attention mass (== Lq).
    Wo_r = Wo64.reshape(E, H, HD)                       # [n, h, b]
    W_eff = lq * np.einsum("ba,nhb->han", Wv64, Wo_r, optimize=True)
    W_eff = W_eff.reshape(E, E).astype(np.float32)      # [k, n]
    b_eff = (lq * np.einsum("nhb,b->n", Wo_r, bv64) + bo64).astype(np.float32)

    # wc[j*P + p, k*P + c] = W_eff[k*P + p, j*P + c]  (lhsT blocks, natural)
    wc = np.ascontiguousarray(
        W_eff.reshape(KT, P, JT, P).transpose(2, 1, 0, 3).reshape(JT * P, E)
    ).astype(np_dtype)
    bw_blk = np.ones((P, JT + P), np.float32)
    bw_blk[:, :JT] = b_eff.reshape(JT, P).T                 # [p, j]

    X = V.reshape(ROWS, E)
    in_maps = []
    for i in range(N_CORES):
        xs_i = np.ascontiguousarray(X[i * RPC:(i + 1) * RPC, :].T).astype(np_dtype)
        head_i = np.empty((P, E + RPC), np_dtype)
        head_i[:, :E] = wc[0:P, :]
        head_i[:, E:] = xs_i[0:P, :]
        in_maps.append({"head": head_i, "xs": xs_i, "wc": wc, "bw": bw_blk})
    return in_maps


def kernel(Q, K, V, Wq, bq, Wk, bk, Wv, bv, Wo, bo, dtype_name="bfloat16",
           n_warm=None, **_unused):
    global LAST_RESULTS
    n, L, e = np.asarray(V).shape
    lq = float(np.asarray(Q).shape[1])
    np_dtype = (np.dtype(ml_dtypes.bfloat16) if dtype_name == "bfloat16"
                else np.float32)
    if n_warm is None:
        n_warm = 6 if dtype_name == "bfloat16" else N_WARM
    in_maps = _prep_in_maps(V, Wv, bv, Wo, bo, lq, np_dtype)
    nc = _get_nc(dtype_name, n_warm)
    LAST_RESULTS = run_bass_kernel_spmd(nc, in_maps, list(range(N_CORES)))
    out = np.concatenate(
        [LAST_RESULTS.results[i]["outT"].T.astype(np.float32)
         for i in range(N_CORES)],
        axis=0,
    )
    return np.ascontiguousarray(out).reshape(n, L, E)

